# revision 27
# baseline (speedup 1.0000x reference)
"""Multi-headed attention (B=4, S=2048, D=512, H=8) on 8 TRN2 NeuronCores.

Sharding: core c handles batch b = c//2 and head-group hg = c%2 (4 of the 8
heads, i.e. a 256-wide slice of the model dim). Each core computes the full
attention for its (batch, 4 heads) and a partial output projection through
the matching 256-column slice of Wo. The host sums the two partials per
batch and adds the output bias.

v3: all matmul operands are fp16 (the PE moving-operand path is
byte-rate-limited, so 16-bit streams 2x faster than f32r; fp16's 11-bit
mantissa keeps quantization error ~8x below bf16 and every intermediate
fits its range: |scores| < 10 so exp < 1e4 < fp16 max). PSUM accumulation
stays fp32. The scalar engine's exp stream is the bottleneck (~1.15us per
[128,1024] strip tile, 128 tiles); all other PE work (projections,
normalize, y-projection) is emitted as filler tasks in the natural PE
idle window between each strip's scores and its attnV (which waits on the
exp), so the exp stream never stalls.

Per-core kernel:
  QT/KT [256, 2048] = W{q,k}T.T @ XT   (e on partitions, seq on free dim)
  V     [2048, 256] = XT.T @ WvT       (+ a ones column per head)
  per (head-pair, q-block 512, k-strip 128):
      scoresT [128, 1024] psum: two row-packed K=64 matmuls (heads at
          partition offsets 0 / 64, concurrent in the PE array)
      exp: one ACTIVATE over the [128, 1024] psum tile -> SBUF fp16
      attnV: per head, [65, 512] psum += V_aug[k,65].T @ expT[k,512]
          (row 64 = ones -> softmax denominator)
  trans: denom row -> SBUF, K=1 matmul broadcast to 64 partitions,
      fast reciprocal, DVE multiply -> atT (normalized, fp16, pairs
      stacked on 128 partitions)
  y [2048, 512] partial: per s-tile, 2 accumulating K=128 matmuls
      (at_pair.T @ WoT_pair) -> copy -> DMA out.
"""

import numpy as np

S = 2048          # sequence length
D = 512           # model dim
EL = 256          # local (per-core) slice of model dim = 4 heads * 64
H = 4             # local heads
DH = 64           # head dim
P = 128           # partitions
NKC = D // P      # k chunks for projections (4)
NST = S // P      # s tiles of 128 (16)
NQB = S // 512    # q blocks of 512 (4)

_CACHE = {}


def _build_nc():
    import concourse.bacc as bacc
    import concourse.mybir as mybir
    import concourse.tile as tile

    F32 = mybir.dt.float32
    F16 = mybir.dt.float16
    EXP = mybir.ActivationFunctionType.Exp

    nc = bacc.Bacc()

    # host-prearranged layouts so every load is contiguous on both the
    # DRAM and SBUF side (4KB/partition descriptor runs, ~2x DMA rate)
    XT = nc.declare_dram_parameter("XT", [4, P, NKC, 512], F16, isOutput=False)
    WQT = nc.declare_dram_parameter("WQT", [P, NKC, EL], F16, isOutput=False)
    WKT = nc.declare_dram_parameter("WKT", [P, NKC, EL], F16, isOutput=False)
    WVT = nc.declare_dram_parameter("WVT", [P, NKC, EL], F16, isOutput=False)
    WOT = nc.declare_dram_parameter("WOT", [P, 2, D], F16, isOutput=False)
    BQ = nc.declare_dram_parameter("BQ", [EL], F32, isOutput=False)
    BK = nc.declare_dram_parameter("BK", [EL], F32, isOutput=False)
    BVB = nc.declare_dram_parameter("BVB", [P, EL], F32, isOutput=False)
    Y = nc.declare_dram_parameter("Y", [S, D], F16, isOutput=True)

    with tile.TileContext(nc) as tc:
        with (
            tc.tile_pool(name="persist", bufs=1) as pp,
            tc.tile_pool(name="pt", bufs=8) as ptp,
            tc.tile_pool(name="rc", bufs=2) as rcp,
            tc.tile_pool(name="rb", bufs=2) as rbp,
            tc.tile_pool(name="ysb", bufs=2) as ysbp,
            tc.tile_pool(name="sc", bufs=2, space="PSUM") as scp,
            tc.tile_pool(name="av", bufs=2, space="PSUM") as avp,
            tc.tile_pool(name="yp", bufs=2, space="PSUM") as ypp,
        ):
            # ---- persistent SBUF tiles ----
            xt = pp.tile([P, 4, NKC, 512], F16)      # X^T, seq-block major
            wqt = pp.tile([P, NKC, EL], F16)
            wkt = pp.tile([P, NKC, EL], F16)
            wvt = pp.tile([P, NKC, EL], F16)
            wot = pp.tile([P, 2, D], F16)            # pairs stacked on parts
            bq = pp.tile([P, 2], F32)
            bk = pp.tile([P, 2], F32)
            bvb = pp.tile([P, EL], F32)
            qt = pp.tile([P, 2, S], F16)             # Q^T (e-chunk on outer)
            kt = pp.tile([P, 2, S], F16)
            vv = pp.tile([P, NST, H, DH + 1], F16)   # V + ones col per head
            at = pp.tile([P, 2, S], F16)             # normalized attn^T,
            #                                          pair on outer, heads
            #                                          stacked on partitions

            # ---- loads, ordered so the first score strip is gated by only
            # ~1MB: wkt + xt seq-chunk 0 + wqt ----
            def xt_chunk(sb):
                nc.sync.dma_start(xt[:, sb], XT[sb])

            for k in range(NKC):
                nc.sync.dma_start(wkt[:, k], WKT[:, k])
                nc.sync.dma_start(xt[:, 0, k], XT[0, :, k])
                nc.sync.dma_start(wqt[:, k], WQT[:, k])
            nc.sync.dma_start(bq[:], BQ.rearrange("(o p) -> p o", p=P))
            nc.sync.dma_start(bk[:], BK.rearrange("(o p) -> p o", p=P))
            nc.sync.dma_start(wvt[:], WVT[:])
            xt_chunk(1)
            nc.sync.dma_start(bvb[:], BVB[:])
            xt_chunk(2)
            xt_chunk(3)
            nc.sync.dma_start(wot[:], WOT[:])

            # ones column per head in the V tile (col DH of each head) -> the
            # attnV matmul's row 64 produces the softmax denominator. Only
            # those columns need the fill; V-proj writes cols 0:DH.
            ones = pp.tile([P, 1, 1, 1], F32)
            nc.vector.memset(ones[:], 1.0)
            nc.vector.tensor_copy(
                vv[:, :, :, DH : DH + 1],
                ones[:].to_broadcast([P, NST, H, 1]),
            )
            # block-indicator lhsT for the per-pair K=33 broadcast matmul:
            # row 0 -> out partitions 0:64 (head A), row 32 -> 64:128 (head
            # B); rows 1..31 are zero (engine APs need 32-aligned partition
            # bases, so the two denominator rows live at partitions 0 and 32)
            ones2 = pp.tile([DH // 2 + 1, P], F16)
            nc.vector.memset(ones2[:], 0.0)
            nc.vector.memset(ones2[0:1, 0:DH], 1.0)
            nc.vector.memset(ones2[DH // 2 : DH // 2 + 1, DH:P], 1.0)
            # persistent ping-pong denominator tile (rows 1..31 stay zero)
            dnp = pp.tile([DH // 2 + 1, 2, 512], F16)
            nc.vector.memset(dnp[:], 0.0)

            # ---- emission helpers ----
            def proj_qk(dst, w, b, e, sb, pool=None, tag="yp"):
                """Project one 512-wide seq block of Q^T or K^T (e-chunk e)."""
                s0 = sb * 512
                ps = (pool or ypp).tile([P, 512], F32, tag=tag)
                for k in range(NKC):
                    nc.tensor.matmul(
                        ps[:],
                        w[:, k, e * P : (e + 1) * P],
                        xt[:, sb, k, :],
                        start=(k == 0),
                        stop=(k == NKC - 1),
                        skip_group_check=True,
                    )
                nc.vector.tensor_scalar_add(
                    dst[:, e, s0 : s0 + 512], ps[:], b[:, e : e + 1]
                )

            def proj_v(st):
                ps = ypp.tile([P, 512], F32, tag="yp")
                for k in range(NKC):
                    nc.tensor.matmul(
                        ps[:, :EL],
                        xt[:, st // 4, k, (st % 4) * P : (st % 4 + 1) * P],
                        wvt[:, k, :],
                        start=(k == 0),
                        stop=(k == NKC - 1),
                        skip_group_check=True,
                    )
                nc.vector.tensor_add(
                    vv[:, st, :, 0:DH],
                    ps[:, :EL].rearrange("p (h d) -> p h d", h=H),
                    bvb[:].rearrange("p (h d) -> p h d", h=H),
                )

            av_store = {}
            dn_store = {}
            pend = []   # deferred attnV emissions: (qb, p, ks, pt)

            def emit_head(qb, p, ks, pt, hp):
                av = av_store[(qb, p)][hp]
                h = 2 * p + hp
                nc.tensor.matmul(
                    av[:],
                    vv[:, ks, h, :],
                    pt[:, hp * 512 : (hp + 1) * 512],
                    start=(ks == 0),
                    stop=(ks == NST - 1),
                    skip_group_check=True,
                )

            # attnV for strip j is emitted `lag` slots later: the exp stream
            # never waits on attnV, and at pair boundaries the extra lag
            # gives the previous pair's normalize (trans) time to release
            # the attnV psum banks before the new pair's first attnV.
            # The two accumulating head-matmuls of each drained strip are
            # split around a slot boundary (head B carried to before the
            # next slot's scores) so two PSUM-accumulating matmuls are
            # never back-to-back: their half-rate read-modify-write drains
            # overlap the fills/drains of the non-accumulating scores.
            LAG = {0: 4, 1: 4, 2: 3, 3: 4, 4: 4, 5: 3,
                   12: 4, 13: 4, 14: 4, 15: 4}
            LAG_LAST = {0: 4, 1: 4, 2: 3, 3: 4, 4: 4, 5: 3, 14: 2, 15: 1}

            def strips(qb, p, sched, lag=LAG):
                """Score + exp (+ lagged attnV) for one (q-block, pair)."""
                q0 = qb * 512
                avA = avp.tile([DH + 1, 512], F32, tag="av")
                avB = avp.tile([DH + 1, 512], F32, tag="av")
                av_store[(qb, p)] = (avA, avB)
                for ks in range(NST):
                    sc = scp.tile([P, 1024], F32, tag="sc")
                    k0 = ks * P
                    nc.tensor.matmul(
                        sc[:, 0:512],
                        kt[0:DH, p, k0 : k0 + P],
                        qt[0:DH, p, q0 : q0 + 512],
                    )
                    nc.tensor.matmul(
                        sc[:, 512:1024],
                        kt[DH:P, p, k0 : k0 + P],
                        qt[DH:P, p, q0 : q0 + 512],
                    )
                    pt = ptp.tile([P, 1024], F16, tag="pt")
                    nc.scalar.activation(pt[:], sc[:], EXP)
                    pend.append((qb, p, ks, pt))
                    while len(pend) > lag.get(ks, 2):
                        e = pend.pop(0)
                        emit_head(*e, 0)
                        emit_head(*e, 1)
                    for task in sched.get(ks, ()):
                        task()

            def flush_attnv():
                while pend:
                    e = pend.pop(0)
                    emit_head(*e, 0)
                    emit_head(*e, 1)

            def trans_dn(qb, p, hp, dn_scalar=False):
                """Stage one head's denominator row into the pair's dn rows
                (partition 0 for head A, partition 32 for head B)."""
                av = av_store[(qb, p)][hp]
                par = (2 * qb + p) % 2
                r = hp * (DH // 2)
                if dn_scalar:
                    nc.scalar.copy(dnp[r : r + 1, par], av[DH : DH + 1, :])
                else:
                    nc.vector.tensor_copy(dnp[r : r + 1, par], av[DH : DH + 1, :])

            def trans_mul(qb, p, dn_scalar=False):
                """Per-pair: bcast both denoms -> recip -> 2 muls into at."""
                q0 = qb * 512
                avA, avB = av_store[(qb, p)]
                par = (2 * qb + p) % 2
                rb_ps = ypp.tile([P, 512], F32, tag="yp")
                nc.tensor.matmul(rb_ps[:], ones2[:], dnp[:, par])
                rb = rbp.tile([P, 512], F32, tag="rb")
                nc.vector.reciprocal_approx_fast(rb[:], rb_ps[:])
                nc.vector.tensor_mul(
                    at[0:DH, p, q0 : q0 + 512], avA[0:DH, :], rb[0:DH, :]
                )
                nc.vector.tensor_mul(
                    at[DH:P, p, q0 : q0 + 512], avB[0:DH, :], rb[DH:P, :]
                )

            def trans(qb, p, hp, dn_scalar=False):
                """Normalize one head of a pair (hp=1 completes the pair)."""
                trans_dn(qb, p, hp, dn_scalar)
                if hp == 1:
                    trans_mul(qb, p, dn_scalar)

            def ytile(st, pool=None, tag="yp", scalar_copy=False):
                yp = (pool or ypp).tile([P, 512], F32, tag=tag)
                for p in range(2):
                    nc.tensor.matmul(
                        yp[:],
                        at[:, p, st * P : (st + 1) * P],
                        wot[:, p, :],
                        start=(p == 0),
                        stop=(p == 1),
                        skip_group_check=True,
                    )
                ysb = ysbp.tile([P, 512], F16, tag="ysb")
                if scalar_copy:
                    nc.scalar.copy(ysb[:], yp[:])
                else:
                    nc.vector.tensor_copy(ysb[:], yp[:])
                nc.sync.dma_start(Y[st * P : (st + 1) * P, :], ysb[:])

            # dummy exp on one element: pulls the one-time ~2.7us activation
            # table load into the initial DMA wait instead of the first real
            # exp on the critical path
            warm = ysbp.tile([P, 2], F32, tag="warm")
            nc.vector.memset(warm[0:1, 0:1], 0.0)
            nc.scalar.activation(warm[0:1, 1:2], warm[0:1, 0:1], EXP)

            # ---- emission order ----
            # upfront: just enough for the first score strip (DMA-gated).
            # The kt and qt block-0 matmul chains are emitted back-to-back
            # (both psum tiles live) so their DVE bias-adds overlap the
            # other chain's matmuls.
            psk = scp.tile([P, 512], F32, tag="sc")
            for k in range(NKC):
                nc.tensor.matmul(
                    psk[:], wkt[:, k, 0:P], xt[:, 0, k, :],
                    start=(k == 0), stop=(k == NKC - 1),
                    skip_group_check=True,
                )
            psq = ypp.tile([P, 512], F32, tag="yp")
            for k in range(NKC):
                nc.tensor.matmul(
                    psq[:], wqt[:, k, 0:P], xt[:, 0, k, :],
                    start=(k == 0), stop=(k == NKC - 1),
                    skip_group_check=True,
                )
            nc.vector.tensor_scalar_add(kt[:, 0, 0:512], psk[:], bk[:, 0:1])
            nc.vector.tensor_scalar_add(qt[:, 0, 0:512], psq[:], bq[:, 0:1])

            T = lambda f, *a: (lambda: f(*a))
            # pair (0,0): V projection rides in every slot; remaining kt
            # blocks arrive a few slots before their first use
            s00 = {ks: [T(proj_v, ks)] for ks in range(NST)}
            s00[1].append(T(proj_qk, kt, wkt, bk, 0, 1))
            s00[5].append(T(proj_qk, kt, wkt, bk, 0, 2))
            s00[9].append(T(proj_qk, kt, wkt, bk, 0, 3))
            s00[11].append(T(proj_qk, kt, wkt, bk, 1, 0))
            s00[13].append(T(proj_qk, qt, wqt, bq, 1, 0))
            strips(0, 0, s00)

            # pair (0,1): normalize (0,0) once its last attnV has been
            # emitted (lagged to slots 0-3), finish kt e1, start qt blocks
            s01 = {
                0: [T(proj_qk, kt, wkt, bk, 1, 1)],
                2: [T(trans_dn, 0, 0, 0)],
                3: [T(trans_dn, 0, 0, 1)],
                4: [T(proj_qk, kt, wkt, bk, 1, 2)],
                5: [T(trans_mul, 0, 0)],
                6: [T(proj_qk, kt, wkt, bk, 1, 3)],
                8: [T(proj_qk, qt, wqt, bq, 0, 1)],
                10: [T(proj_qk, qt, wqt, bq, 1, 1)],
            }
            strips(0, 1, s01)

            # steady state: each pair normalizes the previous pair at slots
            # 2-3; the y tiles of q-block qb-1 are spread over BOTH pairs of
            # qb so heavy and light slots average out
            for qb in range(1, NQB):
                st0 = (qb - 1) * 4
                s0 = {
                    2: [T(trans_dn, qb - 1, 1, 0)],
                    3: [T(trans_dn, qb - 1, 1, 1)],
                    6: [T(trans_mul, qb - 1, 1)],
                    8: [T(ytile, st0)],
                    12: [T(ytile, st0 + 1)],
                }
                if qb == 1:
                    s0[13] = [T(proj_qk, qt, wqt, bq, 0, 2)]
                    s0[14] = [T(proj_qk, qt, wqt, bq, 1, 2)]
                strips(qb, 0, s0)
                s1 = {
                    2: [T(trans_dn, qb, 0, 0)],
                    3: [T(trans_dn, qb, 0, 1)],
                    6: [T(trans_mul, qb, 0)],
                    8: [T(ytile, st0 + 2)],
                    12: [T(ytile, st0 + 3)],
                }
                if qb == 1:
                    s1[13] = [T(proj_qk, qt, wqt, bq, 0, 3)]
                    s1[14] = [T(proj_qk, qt, wqt, bq, 1, 3)]
                strips(qb, 1, s1, lag=LAG_LAST if qb == NQB - 1 else LAG)

            # tail: drain the last two lagged strips so head A's normalize
            # starts as soon as ITS last attnV lands (head B's final attnV
            # overlaps the head-A DVE chain); denominator copies go via the
            # now-idle scalar engine so the vector engine only runs
            # recip+mul; final y tiles run out of the freed attnV psum banks
            # tail: per-head normalize so head A's recip/mul overlap head
            # B's final attnV; denominator copies via the now-idle scalar
            # engine
            def tail_trans(hp):
                qb, p = NQB - 1, 1
                par = (2 * qb + p) % 2
                q0 = qb * 512
                av = av_store[(qb, p)][hp]
                r = hp * (DH // 2)
                rb_ps = ypp.tile([DH, 512], F32, tag="yp")
                nc.tensor.matmul(
                    rb_ps[:],
                    ones2[r : r + 1, hp * DH : (hp + 1) * DH],
                    dnp[r : r + 1, par],
                )
                rb = rbp.tile([DH, 512], F32, tag="rb")
                nc.vector.reciprocal_approx_fast(rb[:], rb_ps[:])
                nc.vector.tensor_mul(
                    at[hp * DH : (hp + 1) * DH, p, q0 : q0 + 512],
                    av[0:DH, :],
                    rb[:],
                )

            e15 = pend.pop(0)
            assert not pend
            emit_head(*e15, 0)
            trans_dn(NQB - 1, 1, 0, dn_scalar=True)
            emit_head(*e15, 1)
            tail_trans(0)
            trans_dn(NQB - 1, 1, 1, dn_scalar=True)
            tail_trans(1)
            for st in range((NQB - 1) * 4, NQB * 4):
                ytile(st, pool=avp, tag="av", scalar_copy=True)

    nc.finalize()
    return nc


def _get_nc():
    if "nc" not in _CACHE:
        _CACHE["nc"] = _build_nc()
    return _CACHE["nc"]


def _prep_inputs(X, Wq, bq, Wk, bk, Wv, bv, Wo, bo):
    F16 = np.float16
    f = lambda a: np.asarray(a, dtype=np.float32)
    X, Wq, bq, Wk, bk, Wv, bv, Wo, bo = map(f, (X, Wq, bq, Wk, bk, Wv, bv, Wo, bo))
    B = X.shape[0]
    scale = np.float32(1.0 / np.sqrt(DH))
    def arr_x(a):  # [D,S] -> [sb, p, o, s]
        return np.ascontiguousarray(
            a.reshape(NKC, P, 4, 512).transpose(2, 1, 0, 3).astype(F16))

    def arr_w(a, o):  # [D_in, E] -> [p, o, e]
        return np.ascontiguousarray(
            a.reshape(o, P, -1).transpose(1, 0, 2).astype(F16))

    XT = [arr_x(X[b].T) for b in range(B)]
    in_maps = []
    for c in range(2 * B):
        b, hg = divmod(c, 2)
        sl = slice(hg * EL, (hg + 1) * EL)
        in_maps.append(
            {
                "XT": XT[b],
                "WQT": arr_w((Wq[sl] * scale).T, NKC),
                "WKT": arr_w(Wk[sl].T, NKC),
                "WVT": arr_w(Wv[sl].T, NKC),
                "WOT": arr_w(Wo[:, sl].T, 2),
                "BQ": np.ascontiguousarray(bq[sl] * scale),
                "BK": np.ascontiguousarray(bk[sl]),
                "BVB": np.ascontiguousarray(np.tile(bv[sl], (P, 1))),
            }
        )
    return in_maps, bo, B


def run(inputs, trace=False, trace_cores=None):
    """Run the kernel; returns (Y_full, exec_time_ns or None)."""
    from concourse.bass_utils import run_bass_kernel_spmd

    in_maps, bo, B = _prep_inputs(**inputs)
    nc = _get_nc()
    kw = {}
    if trace:
        kw = dict(trace=True, trace_cores=trace_cores or list(range(2 * B)))
    res = run_bass_kernel_spmd(nc, in_maps, list(range(2 * B)), **kw)
    Y = np.stack(
        [
            res.results[2 * b]["Y"].astype(np.float32)
            + res.results[2 * b + 1]["Y"].astype(np.float32)
            + bo
            for b in range(B)
        ]
    )
    return Y, getattr(res, "exec_time_ns", None)


def kernel(X, Wq, bq, Wk, bk, Wv, bv, Wo, bo):
    Y, _ = run(
        dict(X=X, Wq=Wq, bq=bq, Wk=Wk, bk=bk, Wv=Wv, bv=bv, Wo=Wo, bo=bo)
    )
    return Y



# revision 31
# speedup vs baseline: 1.0424x; 1.0424x over previous
"""Multi-headed attention (B=4, S=2048, D=512, H=8) on 8 TRN2 NeuronCores.

Sharding: core c handles batch b = c//2 and head-group hg = c%2 (4 of the 8
heads, i.e. a 256-wide slice of the model dim). Each core computes the full
attention for its (batch, 4 heads) and a partial output projection through
the matching 256-column slice of Wo. The host sums the two partials per
batch and adds the output bias.

v3: all matmul operands are fp16 (the PE moving-operand path is
byte-rate-limited, so 16-bit streams 2x faster than f32r; fp16's 11-bit
mantissa keeps quantization error ~8x below bf16 and every intermediate
fits its range: |scores| < 10 so exp < 1e4 < fp16 max). PSUM accumulation
stays fp32. The scalar engine's exp stream is the bottleneck (~1.15us per
[128,1024] strip tile, 128 tiles); all other PE work (projections,
normalize, y-projection) is emitted as filler tasks in the natural PE
idle window between each strip's scores and its attnV (which waits on the
exp), so the exp stream never stalls.

Per-core kernel:
  QT/KT [256, 2048] = W{q,k}T.T @ XT   (e on partitions, seq on free dim)
  V     [2048, 256] = XT.T @ WvT       (+ a ones column per head)
  per (head-pair, q-block 512, k-strip 128):
      scoresT [128, 1024] psum: two row-packed K=64 matmuls (heads at
          partition offsets 0 / 64, concurrent in the PE array)
      exp: one ACTIVATE over the [128, 1024] psum tile -> SBUF fp16
      attnV: per head, [65, 512] psum += V_aug[k,65].T @ expT[k,512]
          (row 64 = ones -> softmax denominator)
  trans: denom row -> SBUF, K=1 matmul broadcast to 64 partitions,
      fast reciprocal, DVE multiply -> atT (normalized, fp16, pairs
      stacked on 128 partitions)
  y [2048, 512] partial: per s-tile, 2 accumulating K=128 matmuls
      (at_pair.T @ WoT_pair) -> copy -> DMA out.
"""

import numpy as np

S = 2048          # sequence length
D = 512           # model dim
EL = 256          # local (per-core) slice of model dim = 4 heads * 64
H = 4             # local heads
DH = 64           # head dim
P = 128           # partitions
NKC = D // P      # k chunks for projections (4)
NST = S // P      # s tiles of 128 (16)
NQB = S // 512    # q blocks of 512 (4)

_CACHE = {}


def _build_nc():
    import concourse.bacc as bacc
    import concourse.mybir as mybir
    import concourse.tile as tile

    F32 = mybir.dt.float32
    F16 = mybir.dt.float16
    EXP = mybir.ActivationFunctionType.Exp

    nc = bacc.Bacc()

    # host-prearranged layouts so every load is contiguous on both the
    # DRAM and SBUF side (4KB/partition descriptor runs, ~2x DMA rate)
    XT = nc.declare_dram_parameter("XT", [4, P, NKC, 512], F16, isOutput=False)
    WQT = nc.declare_dram_parameter("WQT", [P, NKC, EL], F16, isOutput=False)
    WKT = nc.declare_dram_parameter("WKT", [P, NKC, EL], F16, isOutput=False)
    WVT = nc.declare_dram_parameter("WVT", [P, NKC, EL], F16, isOutput=False)
    WOT = nc.declare_dram_parameter("WOT", [P, 2, D], F16, isOutput=False)
    BQ = nc.declare_dram_parameter("BQ", [EL], F32, isOutput=False)
    BK = nc.declare_dram_parameter("BK", [EL], F32, isOutput=False)
    BVB = nc.declare_dram_parameter("BVB", [P, EL], F32, isOutput=False)
    Y = nc.declare_dram_parameter("Y", [S, D], F16, isOutput=True)

    with tile.TileContext(nc) as tc:
        with (
            tc.tile_pool(name="persist", bufs=1) as pp,
            tc.tile_pool(name="pt", bufs=8) as ptp,
            tc.tile_pool(name="rc", bufs=2) as rcp,
            tc.tile_pool(name="rb", bufs=2) as rbp,
            tc.tile_pool(name="ysb", bufs=2) as ysbp,
            tc.tile_pool(name="sc", bufs=2, space="PSUM") as scp,
            tc.tile_pool(name="av", bufs=2, space="PSUM") as avp,
            tc.tile_pool(name="yp", bufs=2, space="PSUM") as ypp,
        ):
            # ---- persistent SBUF tiles ----
            xt = pp.tile([P, 4, NKC, 512], F16)      # X^T, seq-block major
            wqt = pp.tile([P, NKC, EL], F16)
            wkt = pp.tile([P, NKC, EL], F16)
            wvt = pp.tile([P, NKC, EL], F16)
            wot = pp.tile([P, 2, D], F16)            # pairs stacked on parts
            bq = pp.tile([P, 2], F32)
            bk = pp.tile([P, 2], F32)
            bvb = pp.tile([P, EL], F32)
            qt = pp.tile([P, 2, S], F16)             # Q^T (e-chunk on outer)
            kt = pp.tile([P, 2, S], F16)
            vv = pp.tile([P, NST, H, DH + 1], F16)   # V + ones col per head
            at = pp.tile([P, 2, S], F16)             # normalized attn^T,
            #                                          pair on outer, heads
            #                                          stacked on partitions

            # ---- loads, ordered so the first score strip is gated by only
            # ~1MB: wkt + xt seq-chunk 0 + wqt ----
            def xt_chunk(sb):
                nc.sync.dma_start(xt[:, sb], XT[sb])

            nc.sync.dma_start(wkt[:], WKT[:])
            xt_chunk(0)
            nc.sync.dma_start(wqt[:], WQT[:])
            nc.sync.dma_start(bq[:], BQ.rearrange("(o p) -> p o", p=P))
            nc.sync.dma_start(bk[:], BK.rearrange("(o p) -> p o", p=P))
            xt_chunk(1)
            nc.sync.dma_start(wvt[:], WVT[:])
            xt_chunk(2)
            xt_chunk(3)
            nc.sync.dma_start(bvb[:], BVB[:])
            nc.sync.dma_start(wot[:], WOT[:])

            # ones column per head in the V tile (col DH of each head) -> the
            # attnV matmul's row 64 produces the softmax denominator. Only
            # those columns need the fill; V-proj writes cols 0:DH.
            ones = pp.tile([P, 1, 1, 1], F32)
            nc.vector.memset(ones[:], 1.0)
            nc.vector.tensor_copy(
                vv[:, :, :, DH : DH + 1],
                ones[:].to_broadcast([P, NST, H, 1]),
            )
            # block-indicator lhsT for the per-pair K=33 broadcast matmul:
            # row 0 -> out partitions 0:64 (head A), row 32 -> 64:128 (head
            # B); rows 1..31 are zero (engine APs need 32-aligned partition
            # bases, so the two denominator rows live at partitions 0 and 32)
            ones2 = pp.tile([DH // 2 + 1, P], F16)
            nc.vector.memset(ones2[:], 0.0)
            nc.vector.memset(ones2[0:1, 0:DH], 1.0)
            nc.vector.memset(ones2[DH // 2 : DH // 2 + 1, DH:P], 1.0)
            # persistent ping-pong denominator tile (rows 1..31 stay zero)
            dnp = pp.tile([DH // 2 + 1, 2, 512], F16)
            nc.vector.memset(dnp[:], 0.0)

            # ---- emission helpers ----
            def proj_qk(dst, w, b, e, sb, pool=None, tag="yp"):
                """Project one 512-wide seq block of Q^T or K^T (e-chunk e)."""
                s0 = sb * 512
                ps = (pool or ypp).tile([P, 512], F32, tag=tag)
                for k in range(NKC):
                    nc.tensor.matmul(
                        ps[:],
                        w[:, k, e * P : (e + 1) * P],
                        xt[:, sb, k, :],
                        start=(k == 0),
                        stop=(k == NKC - 1),
                        skip_group_check=True,
                    )
                nc.vector.tensor_scalar_add(
                    dst[:, e, s0 : s0 + 512], ps[:], b[:, e : e + 1]
                )

            def proj_v(st):
                ps = ypp.tile([P, 512], F32, tag="yp")
                for k in range(NKC):
                    nc.tensor.matmul(
                        ps[:, :EL],
                        xt[:, st // 4, k, (st % 4) * P : (st % 4 + 1) * P],
                        wvt[:, k, :],
                        start=(k == 0),
                        stop=(k == NKC - 1),
                        skip_group_check=True,
                    )
                nc.vector.tensor_add(
                    vv[:, st, :, 0:DH],
                    ps[:, :EL].rearrange("p (h d) -> p h d", h=H),
                    bvb[:].rearrange("p (h d) -> p h d", h=H),
                )

            av_store = {}
            dn_store = {}
            pend = []   # deferred attnV emissions: (qb, p, ks, pt)

            def emit_head(qb, p, ks, pt, hp):
                av = av_store[(qb, p)][hp]
                h = 2 * p + hp
                nc.tensor.matmul(
                    av[:],
                    vv[:, ks, h, :],
                    pt[:, hp * 512 : (hp + 1) * 512],
                    start=(ks == 0),
                    stop=(ks == NST - 1),
                    skip_group_check=True,
                )

            # attnV for strip j is emitted `lag` slots later: the exp stream
            # never waits on attnV, and at pair boundaries the extra lag
            # gives the previous pair's normalize (trans) time to release
            # the attnV psum banks before the new pair's first attnV.
            # The two accumulating head-matmuls of each drained strip are
            # split around a slot boundary (head B carried to before the
            # next slot's scores) so two PSUM-accumulating matmuls are
            # never back-to-back: their half-rate read-modify-write drains
            # overlap the fills/drains of the non-accumulating scores.
            # constant lag -> exactly one strip's attnV pair drains per slot
            # (bunched drains of accumulating matmuls run at RMW half-rate
            # and starve the exp stream)
            LAG = {ks: 4 for ks in range(NST)}
            LAG_LAST = {**{ks: 4 for ks in range(12)}, 12: 3, 13: 2, 14: 1, 15: 1}

            def strips(qb, p, sched, lag=LAG):
                """Score + exp (+ lagged attnV) for one (q-block, pair)."""
                q0 = qb * 512
                avA = avp.tile([DH + 1, 512], F32, tag="av")
                avB = avp.tile([DH + 1, 512], F32, tag="av")
                av_store[(qb, p)] = (avA, avB)
                for ks in range(NST):
                    sc = scp.tile([P, 1024], F32, tag="sc")
                    k0 = ks * P
                    nc.tensor.matmul(
                        sc[:, 0:512],
                        kt[0:DH, p, k0 : k0 + P],
                        qt[0:DH, p, q0 : q0 + 512],
                    )
                    nc.tensor.matmul(
                        sc[:, 512:1024],
                        kt[DH:P, p, k0 : k0 + P],
                        qt[DH:P, p, q0 : q0 + 512],
                    )
                    pt = ptp.tile([P, 1024], F16, tag="pt")
                    nc.scalar.activation(pt[:], sc[:], EXP)
                    pend.append((qb, p, ks, pt))
                    while len(pend) > lag.get(ks, 2):
                        e = pend.pop(0)
                        emit_head(*e, 0)
                        emit_head(*e, 1)
                    for task in sched.get(ks, ()):
                        task()

            def flush_attnv():
                while pend:
                    e = pend.pop(0)
                    emit_head(*e, 0)
                    emit_head(*e, 1)

            def trans_dn(qb, p, hp, dn_scalar=False):
                """Stage one head's denominator row into the pair's dn rows
                (partition 0 for head A, partition 32 for head B)."""
                av = av_store[(qb, p)][hp]
                par = (2 * qb + p) % 2
                r = hp * (DH // 2)
                if dn_scalar:
                    nc.scalar.copy(dnp[r : r + 1, par], av[DH : DH + 1, :])
                else:
                    nc.vector.tensor_copy(dnp[r : r + 1, par], av[DH : DH + 1, :])

            def trans_mul(qb, p, dn_scalar=False):
                """Per-pair: bcast both denoms -> recip -> 2 muls into at."""
                q0 = qb * 512
                avA, avB = av_store[(qb, p)]
                par = (2 * qb + p) % 2
                rb_ps = ypp.tile([P, 512], F32, tag="yp")
                nc.tensor.matmul(rb_ps[:], ones2[:], dnp[:, par])
                rb = rbp.tile([P, 512], F32, tag="rb")
                nc.vector.reciprocal_approx_fast(rb[:], rb_ps[:])
                nc.vector.tensor_mul(
                    at[0:DH, p, q0 : q0 + 512], avA[0:DH, :], rb[0:DH, :]
                )
                nc.vector.tensor_mul(
                    at[DH:P, p, q0 : q0 + 512], avB[0:DH, :], rb[DH:P, :]
                )

            def trans(qb, p, hp, dn_scalar=False):
                """Normalize one head of a pair (hp=1 completes the pair)."""
                trans_dn(qb, p, hp, dn_scalar)
                if hp == 1:
                    trans_mul(qb, p, dn_scalar)

            def ytile(st, pool=None, tag="yp", scalar_copy=False):
                yp = (pool or ypp).tile([P, 512], F32, tag=tag)
                for p in range(2):
                    nc.tensor.matmul(
                        yp[:],
                        at[:, p, st * P : (st + 1) * P],
                        wot[:, p, :],
                        start=(p == 0),
                        stop=(p == 1),
                        skip_group_check=True,
                    )
                ysb = ysbp.tile([P, 512], F16, tag="ysb")
                if scalar_copy:
                    nc.scalar.copy(ysb[:], yp[:])
                else:
                    nc.vector.tensor_copy(ysb[:], yp[:])
                nc.sync.dma_start(Y[st * P : (st + 1) * P, :], ysb[:])

            # dummy exp on one element: pulls the one-time ~2.7us activation
            # table load into the initial DMA wait instead of the first real
            # exp on the critical path
            warm = ysbp.tile([P, 2], F32, tag="warm")
            nc.vector.memset(warm[0:1, 0:1], 0.0)
            nc.scalar.activation(warm[0:1, 1:2], warm[0:1, 0:1], EXP)

            # ---- emission order ----
            # upfront: just enough for the first score strip (DMA-gated).
            # The kt and qt block-0 matmul chains are emitted back-to-back
            # (both psum tiles live) so their DVE bias-adds overlap the
            # other chain's matmuls.
            psk = scp.tile([P, 512], F32, tag="sc")
            for k in range(NKC):
                nc.tensor.matmul(
                    psk[:], wkt[:, k, 0:P], xt[:, 0, k, :],
                    start=(k == 0), stop=(k == NKC - 1),
                    skip_group_check=True,
                )
            psq = ypp.tile([P, 512], F32, tag="yp")
            for k in range(NKC):
                nc.tensor.matmul(
                    psq[:], wqt[:, k, 0:P], xt[:, 0, k, :],
                    start=(k == 0), stop=(k == NKC - 1),
                    skip_group_check=True,
                )
            nc.vector.tensor_scalar_add(kt[:, 0, 0:512], psk[:], bk[:, 0:1])
            nc.vector.tensor_scalar_add(qt[:, 0, 0:512], psq[:], bq[:, 0:1])

            T = lambda f, *a: (lambda: f(*a))
            # pair (0,0): V projection rides in every slot; remaining kt
            # blocks arrive a few slots before their first use
            s00 = {ks: [T(proj_v, ks)] for ks in range(NST)}
            s00[1].append(T(proj_qk, kt, wkt, bk, 0, 1))
            s00[5].append(T(proj_qk, kt, wkt, bk, 0, 2))
            s00[9].append(T(proj_qk, kt, wkt, bk, 0, 3))
            s00[11].append(T(proj_qk, kt, wkt, bk, 1, 0))
            s00[13].append(T(proj_qk, qt, wqt, bq, 1, 0))
            strips(0, 0, s00)

            # pair (0,1): normalize (0,0) once its last attnV has been
            # emitted (lagged to slots 0-3), finish kt e1, start qt blocks
            s01 = {
                0: [T(proj_qk, kt, wkt, bk, 1, 1)],
                2: [T(proj_qk, kt, wkt, bk, 1, 2)],
                4: [T(trans_dn, 0, 0, 0)],
                5: [T(trans_dn, 0, 0, 1)],
                6: [T(trans_mul, 0, 0)],
                7: [T(proj_qk, kt, wkt, bk, 1, 3)],
                8: [T(proj_qk, qt, wqt, bq, 0, 1)],
                10: [T(proj_qk, qt, wqt, bq, 1, 1)],
            }
            strips(0, 1, s01)

            # steady state: each pair normalizes the previous pair at slots
            # 2-3; the y tiles of q-block qb-1 are spread over BOTH pairs of
            # qb so heavy and light slots average out
            for qb in range(1, NQB):
                st0 = (qb - 1) * 4
                s0 = {
                    4: [T(trans_dn, qb - 1, 1, 0)],
                    5: [T(trans_dn, qb - 1, 1, 1)],
                    6: [T(trans_mul, qb - 1, 1)],
                    8: [T(ytile, st0)],
                    12: [T(ytile, st0 + 1)],
                }
                if qb == 1:
                    s0[13] = [T(proj_qk, qt, wqt, bq, 0, 2)]
                    s0[14] = [T(proj_qk, qt, wqt, bq, 1, 2)]
                strips(qb, 0, s0)
                s1 = {
                    4: [T(trans_dn, qb, 0, 0)],
                    5: [T(trans_dn, qb, 0, 1)],
                    6: [T(trans_mul, qb, 0)],
                    8: [T(ytile, st0 + 2)],
                    12: [T(ytile, st0 + 3)],
                }
                if qb == 1:
                    s1[13] = [T(proj_qk, qt, wqt, bq, 0, 3)]
                    s1[14] = [T(proj_qk, qt, wqt, bq, 1, 3)]
                strips(qb, 1, s1, lag=LAG_LAST if qb == NQB - 1 else LAG)

            # tail: drain the last two lagged strips so head A's normalize
            # starts as soon as ITS last attnV lands (head B's final attnV
            # overlaps the head-A DVE chain); denominator copies go via the
            # now-idle scalar engine so the vector engine only runs
            # recip+mul; final y tiles run out of the freed attnV psum banks
            # tail: per-head normalize so head A's recip/mul overlap head
            # B's final attnV; denominator copies via the now-idle scalar
            # engine
            def tail_trans(hp):
                qb, p = NQB - 1, 1
                par = (2 * qb + p) % 2
                q0 = qb * 512
                av = av_store[(qb, p)][hp]
                r = hp * (DH // 2)
                rb_ps = ypp.tile([DH, 512], F32, tag="yp")
                nc.tensor.matmul(
                    rb_ps[:],
                    ones2[r : r + 1, hp * DH : (hp + 1) * DH],
                    dnp[r : r + 1, par],
                )
                rb = rbp.tile([DH, 512], F32, tag="rb")
                nc.vector.reciprocal_approx_fast(rb[:], rb_ps[:])
                nc.vector.tensor_mul(
                    at[hp * DH : (hp + 1) * DH, p, q0 : q0 + 512],
                    av[0:DH, :],
                    rb[:],
                )

            e15 = pend.pop(0)
            assert not pend
            emit_head(*e15, 0)
            trans_dn(NQB - 1, 1, 0, dn_scalar=True)
            emit_head(*e15, 1)
            tail_trans(0)
            trans_dn(NQB - 1, 1, 1, dn_scalar=True)
            tail_trans(1)
            for st in range((NQB - 1) * 4, NQB * 4):
                ytile(st, pool=avp, tag="av", scalar_copy=True)

    nc.finalize()
    return nc


def _get_nc():
    if "nc" not in _CACHE:
        _CACHE["nc"] = _build_nc()
    return _CACHE["nc"]


def _prep_inputs(X, Wq, bq, Wk, bk, Wv, bv, Wo, bo):
    F16 = np.float16
    f = lambda a: np.asarray(a, dtype=np.float32)
    X, Wq, bq, Wk, bk, Wv, bv, Wo, bo = map(f, (X, Wq, bq, Wk, bk, Wv, bv, Wo, bo))
    B = X.shape[0]
    scale = np.float32(1.0 / np.sqrt(DH))
    def arr_x(a):  # [D,S] -> [sb, p, o, s]
        return np.ascontiguousarray(
            a.reshape(NKC, P, 4, 512).transpose(2, 1, 0, 3).astype(F16))

    def arr_w(a, o):  # [D_in, E] -> [p, o, e]
        return np.ascontiguousarray(
            a.reshape(o, P, -1).transpose(1, 0, 2).astype(F16))

    XT = [arr_x(X[b].T) for b in range(B)]
    in_maps = []
    for c in range(2 * B):
        b, hg = divmod(c, 2)
        sl = slice(hg * EL, (hg + 1) * EL)
        in_maps.append(
            {
                "XT": XT[b],
                "WQT": arr_w((Wq[sl] * scale).T, NKC),
                "WKT": arr_w(Wk[sl].T, NKC),
                "WVT": arr_w(Wv[sl].T, NKC),
                "WOT": arr_w(Wo[:, sl].T, 2),
                "BQ": np.ascontiguousarray(bq[sl] * scale),
                "BK": np.ascontiguousarray(bk[sl]),
                "BVB": np.ascontiguousarray(np.tile(bv[sl], (P, 1))),
            }
        )
    return in_maps, bo, B


def run(inputs, trace=False, trace_cores=None):
    """Run the kernel; returns (Y_full, exec_time_ns or None)."""
    from concourse.bass_utils import run_bass_kernel_spmd

    in_maps, bo, B = _prep_inputs(**inputs)
    nc = _get_nc()
    kw = {}
    if trace:
        kw = dict(trace=True, trace_cores=trace_cores or list(range(2 * B)))
    res = run_bass_kernel_spmd(nc, in_maps, list(range(2 * B)), **kw)
    Y = np.stack(
        [
            res.results[2 * b]["Y"].astype(np.float32)
            + res.results[2 * b + 1]["Y"].astype(np.float32)
            + bo
            for b in range(B)
        ]
    )
    return Y, getattr(res, "exec_time_ns", None)


def kernel(X, Wq, bq, Wk, bk, Wv, bv, Wo, bo):
    Y, _ = run(
        dict(X=X, Wq=Wq, bq=bq, Wk=Wk, bk=bk, Wv=Wv, bv=bv, Wo=Wo, bo=bo)
    )
    return Y



# revision 36
# speedup vs baseline: 1.0491x; 1.0064x over previous
"""Multi-headed attention (B=4, S=2048, D=512, H=8) on 8 TRN2 NeuronCores.

Sharding: core c handles batch b = c//2 and head-group hg = c%2 (4 of the 8
heads, i.e. a 256-wide slice of the model dim). Each core computes the full
attention for its (batch, 4 heads) and a partial output projection through
the matching 256-column slice of Wo. The host sums the two partials per
batch and adds the output bias.

v3: all matmul operands are fp16 (the PE moving-operand path is
byte-rate-limited, so 16-bit streams 2x faster than f32r; fp16's 11-bit
mantissa keeps quantization error ~8x below bf16 and every intermediate
fits its range: |scores| < 10 so exp < 1e4 < fp16 max). PSUM accumulation
stays fp32. The scalar engine's exp stream is the bottleneck (~1.15us per
[128,1024] strip tile, 128 tiles); all other PE work (projections,
normalize, y-projection) is emitted as filler tasks in the natural PE
idle window between each strip's scores and its attnV (which waits on the
exp), so the exp stream never stalls.

Per-core kernel:
  QT/KT [256, 2048] = W{q,k}T.T @ XT   (e on partitions, seq on free dim)
  V     [2048, 256] = XT.T @ WvT       (+ a ones column per head)
  per (head-pair, q-block 512, k-strip 128):
      scoresT [128, 1024] psum: two row-packed K=64 matmuls (heads at
          partition offsets 0 / 64, concurrent in the PE array)
      exp: one ACTIVATE over the [128, 1024] psum tile -> SBUF fp16
      attnV: per head, [65, 512] psum += V_aug[k,65].T @ expT[k,512]
          (row 64 = ones -> softmax denominator)
  trans: denom row -> SBUF, K=1 matmul broadcast to 64 partitions,
      fast reciprocal, DVE multiply -> atT (normalized, fp16, pairs
      stacked on 128 partitions)
  y [2048, 512] partial: per s-tile, 2 accumulating K=128 matmuls
      (at_pair.T @ WoT_pair) -> copy -> DMA out.
"""

import numpy as np

S = 2048          # sequence length
D = 512           # model dim
EL = 256          # local (per-core) slice of model dim = 4 heads * 64
H = 4             # local heads
DH = 64           # head dim
P = 128           # partitions
NKC = D // P      # k chunks for projections (4)
NST = S // P      # s tiles of 128 (16)
NQB = S // 512    # q blocks of 512 (4)

_CACHE = {}


def _build_nc():
    import concourse.bacc as bacc
    import concourse.mybir as mybir
    import concourse.tile as tile

    F32 = mybir.dt.float32
    F16 = mybir.dt.float16
    EXP = mybir.ActivationFunctionType.Exp

    nc = bacc.Bacc()

    # host-prearranged layouts so every load is contiguous on both the
    # DRAM and SBUF side (4KB/partition descriptor runs, ~2x DMA rate)
    XT = nc.declare_dram_parameter("XT", [4, P, NKC, 512], F16, isOutput=False)
    WQT = nc.declare_dram_parameter("WQT", [P, NKC, EL], F16, isOutput=False)
    WKT = nc.declare_dram_parameter("WKT", [P, NKC, EL], F16, isOutput=False)
    WVT = nc.declare_dram_parameter("WVT", [P, NKC, EL], F16, isOutput=False)
    WOT = nc.declare_dram_parameter("WOT", [P, 2, D], F16, isOutput=False)
    BQ = nc.declare_dram_parameter("BQ", [EL], F32, isOutput=False)
    BK = nc.declare_dram_parameter("BK", [EL], F32, isOutput=False)
    BVB = nc.declare_dram_parameter("BVB", [P, EL], F32, isOutput=False)
    Y = nc.declare_dram_parameter("Y", [S, D], F16, isOutput=True)

    with tile.TileContext(nc) as tc:
        with (
            tc.tile_pool(name="persist", bufs=1) as pp,
            tc.tile_pool(name="pt", bufs=8) as ptp,
            tc.tile_pool(name="rc", bufs=2) as rcp,
            tc.tile_pool(name="rb", bufs=2) as rbp,
            tc.tile_pool(name="ysb", bufs=2) as ysbp,
            tc.tile_pool(name="sc", bufs=2, space="PSUM") as scp,
            tc.tile_pool(name="av", bufs=2, space="PSUM") as avp,
            tc.tile_pool(name="yp", bufs=2, space="PSUM") as ypp,
        ):
            # ---- persistent SBUF tiles ----
            xt = pp.tile([P, 4, NKC, 512], F16)      # X^T, seq-block major
            wqt = pp.tile([P, NKC, EL], F16)
            wkt = pp.tile([P, NKC, EL], F16)
            wvt = pp.tile([P, NKC, EL], F16)
            wot = pp.tile([P, 2, D], F16)            # pairs stacked on parts
            bq = pp.tile([P, 2], F32)
            bk = pp.tile([P, 2], F32)
            bvb = pp.tile([P, EL], F32)
            qt = pp.tile([P, 2, S], F16)             # Q^T (e-chunk on outer)
            kt = pp.tile([P, 2, S], F16)
            vv = pp.tile([P, NST, H, DH + 1], F16)   # V + ones col per head
            at = pp.tile([P, 2, S], F16)             # normalized attn^T,
            #                                          pair on outer, heads
            #                                          stacked on partitions

            # ---- loads, ordered so the first score strip is gated by only
            # ~1MB: wkt + xt seq-chunk 0 + wqt ----
            def xt_chunk(sb):
                nc.sync.dma_start(xt[:, sb], XT[sb])

            nc.sync.dma_start(wkt[:], WKT[:])
            xt_chunk(0)
            nc.sync.dma_start(wqt[:], WQT[:])
            nc.sync.dma_start(bq[:], BQ.rearrange("(o p) -> p o", p=P))
            nc.sync.dma_start(bk[:], BK.rearrange("(o p) -> p o", p=P))
            xt_chunk(1)
            nc.sync.dma_start(wvt[:], WVT[:])
            xt_chunk(2)
            xt_chunk(3)
            nc.sync.dma_start(bvb[:], BVB[:])
            nc.sync.dma_start(wot[:], WOT[:])

            # ones column per head in the V tile (col DH of each head) -> the
            # attnV matmul's row 64 produces the softmax denominator. Only
            # those columns need the fill; V-proj writes cols 0:DH.
            ones = pp.tile([P, 1, 1, 1], F32)
            nc.vector.memset(ones[:], 1.0)
            nc.vector.tensor_copy(
                vv[:, :, :, DH : DH + 1],
                ones[:].to_broadcast([P, NST, H, 1]),
            )
            # block-indicator lhsT for the per-pair K=33 broadcast matmul:
            # row 0 -> out partitions 0:64 (head A), row 32 -> 64:128 (head
            # B); rows 1..31 are zero (engine APs need 32-aligned partition
            # bases, so the two denominator rows live at partitions 0 and 32)
            ones2 = pp.tile([DH // 2 + 1, P], F16)
            nc.vector.memset(ones2[:], 0.0)
            nc.vector.memset(ones2[0:1, 0:DH], 1.0)
            nc.vector.memset(ones2[DH // 2 : DH // 2 + 1, DH:P], 1.0)
            # persistent ping-pong denominator tile (rows 1..31 stay zero)
            dnp = pp.tile([DH // 2 + 1, 2, 512], F16)
            nc.vector.memset(dnp[:], 0.0)

            # ---- emission helpers ----
            pq_ps = {}

            def proj_qk_h(dst, w, b, e, sb, half, pool=None, tag="yp"):
                """Half of a Q^T/K^T block projection: 2 of 4 k-chunk
                matmuls. half=0 allocates the psum tile; half=1 finishes the
                chain and applies the bias-add drain."""
                s0 = sb * 512
                key = (id(dst), e, sb)
                if half == 0:
                    ps = (pool or ypp).tile([P, 512], F32, tag=tag)
                    pq_ps[key] = ps
                else:
                    ps = pq_ps.pop(key)
                for k in (2 * half, 2 * half + 1):
                    nc.tensor.matmul(
                        ps[:],
                        w[:, k, e * P : (e + 1) * P],
                        xt[:, sb, k, :],
                        start=(k == 0),
                        stop=(k == NKC - 1),
                        skip_group_check=True,
                    )
                if half == 1:
                    nc.vector.tensor_scalar_add(
                        dst[:, e, s0 : s0 + 512], ps[:], b[:, e : e + 1]
                    )

            def proj_qk(dst, w, b, e, sb, pool=None, tag="yp"):
                """Project one 512-wide seq block of Q^T or K^T (e-chunk e)."""
                proj_qk_h(dst, w, b, e, sb, 0, pool=pool, tag=tag)
                proj_qk_h(dst, w, b, e, sb, 1, pool=pool, tag=tag)

            def proj_v(st):
                ps = ypp.tile([P, 512], F32, tag="yp")
                for k in range(NKC):
                    nc.tensor.matmul(
                        ps[:, :EL],
                        xt[:, st // 4, k, (st % 4) * P : (st % 4 + 1) * P],
                        wvt[:, k, :],
                        start=(k == 0),
                        stop=(k == NKC - 1),
                        skip_group_check=True,
                    )
                nc.vector.tensor_add(
                    vv[:, st, :, 0:DH],
                    ps[:, :EL].rearrange("p (h d) -> p h d", h=H),
                    bvb[:].rearrange("p (h d) -> p h d", h=H),
                )

            av_store = {}
            dn_store = {}
            pend = []   # deferred attnV emissions: (qb, p, ks, pt)

            def emit_head(qb, p, ks, pt, hp):
                av = av_store[(qb, p)][hp]
                h = 2 * p + hp
                nc.tensor.matmul(
                    av[:],
                    vv[:, ks, h, :],
                    pt[:, hp * 512 : (hp + 1) * 512],
                    start=(ks == 0),
                    stop=(ks == NST - 1),
                    skip_group_check=True,
                )

            # attnV for strip j is emitted `lag` slots later: the exp stream
            # never waits on attnV, and at pair boundaries the extra lag
            # gives the previous pair's normalize (trans) time to release
            # the attnV psum banks before the new pair's first attnV.
            # The two accumulating head-matmuls of each drained strip are
            # split around a slot boundary (head B carried to before the
            # next slot's scores) so two PSUM-accumulating matmuls are
            # never back-to-back: their half-rate read-modify-write drains
            # overlap the fills/drains of the non-accumulating scores.
            # constant lag -> exactly one strip's attnV pair drains per slot
            # (bunched drains of accumulating matmuls run at RMW half-rate
            # and starve the exp stream)
            LAG = {ks: 4 for ks in range(NST)}
            LAG_LAST = {**{ks: 4 for ks in range(12)}, 12: 3, 13: 2, 14: 1, 15: 1}

            def strips(qb, p, sched, lag=LAG):
                """Score + exp (+ lagged attnV) for one (q-block, pair)."""
                q0 = qb * 512
                avA = avp.tile([DH + 1, 512], F32, tag="av")
                avB = avp.tile([DH + 1, 512], F32, tag="av")
                av_store[(qb, p)] = (avA, avB)
                for ks in range(NST):
                    sc = scp.tile([P, 1024], F32, tag="sc")
                    k0 = ks * P
                    nc.tensor.matmul(
                        sc[:, 0:512],
                        kt[0:DH, p, k0 : k0 + P],
                        qt[0:DH, p, q0 : q0 + 512],
                    )
                    nc.tensor.matmul(
                        sc[:, 512:1024],
                        kt[DH:P, p, k0 : k0 + P],
                        qt[DH:P, p, q0 : q0 + 512],
                    )
                    pt = ptp.tile([P, 1024], F16, tag="pt")
                    nc.scalar.activation(pt[:], sc[:], EXP)
                    pend.append((qb, p, ks, pt))
                    while len(pend) > lag.get(ks, 2):
                        e = pend.pop(0)
                        emit_head(*e, 0)
                        emit_head(*e, 1)
                    for task in sched.get(ks, ()):
                        task()

            def flush_attnv():
                while pend:
                    e = pend.pop(0)
                    emit_head(*e, 0)
                    emit_head(*e, 1)

            def trans_dn(qb, p, hp, dn_scalar=False):
                """Stage one head's denominator row into the pair's dn rows
                (partition 0 for head A, partition 32 for head B)."""
                av = av_store[(qb, p)][hp]
                par = (2 * qb + p) % 2
                r = hp * (DH // 2)
                if dn_scalar:
                    nc.scalar.copy(dnp[r : r + 1, par], av[DH : DH + 1, :])
                else:
                    nc.vector.tensor_copy(dnp[r : r + 1, par], av[DH : DH + 1, :])

            def trans_mul(qb, p, dn_scalar=False):
                """Per-pair: bcast both denoms -> recip -> 2 muls into at."""
                q0 = qb * 512
                avA, avB = av_store[(qb, p)]
                par = (2 * qb + p) % 2
                rb_ps = ypp.tile([P, 512], F32, tag="yp")
                nc.tensor.matmul(rb_ps[:], ones2[:], dnp[:, par])
                rb = rbp.tile([P, 512], F32, tag="rb")
                nc.vector.reciprocal_approx_fast(rb[:], rb_ps[:])
                nc.vector.tensor_mul(
                    at[0:DH, p, q0 : q0 + 512], avA[0:DH, :], rb[0:DH, :]
                )
                nc.vector.tensor_mul(
                    at[DH:P, p, q0 : q0 + 512], avB[0:DH, :], rb[DH:P, :]
                )

            def trans(qb, p, hp, dn_scalar=False):
                """Normalize one head of a pair (hp=1 completes the pair)."""
                trans_dn(qb, p, hp, dn_scalar)
                if hp == 1:
                    trans_mul(qb, p, dn_scalar)

            def ytile(st, pool=None, tag="yp", scalar_copy=False):
                yp = (pool or ypp).tile([P, 512], F32, tag=tag)
                for p in range(2):
                    nc.tensor.matmul(
                        yp[:],
                        at[:, p, st * P : (st + 1) * P],
                        wot[:, p, :],
                        start=(p == 0),
                        stop=(p == 1),
                        skip_group_check=True,
                    )
                ysb = ysbp.tile([P, 512], F16, tag="ysb")
                if scalar_copy:
                    nc.scalar.copy(ysb[:], yp[:])
                else:
                    nc.vector.tensor_copy(ysb[:], yp[:])
                nc.sync.dma_start(Y[st * P : (st + 1) * P, :], ysb[:])

            # dummy exp on one element: pulls the one-time ~2.7us activation
            # table load into the initial DMA wait instead of the first real
            # exp on the critical path
            warm = ysbp.tile([P, 2], F32, tag="warm")
            nc.vector.memset(warm[0:1, 0:1], 0.0)
            nc.scalar.activation(warm[0:1, 1:2], warm[0:1, 0:1], EXP)

            # ---- emission order ----
            # upfront: just enough for the first score strip (DMA-gated).
            # The kt and qt block-0 matmul chains are emitted back-to-back
            # (both psum tiles live) so their DVE bias-adds overlap the
            # other chain's matmuls.
            psk = scp.tile([P, 512], F32, tag="sc")
            for k in range(NKC):
                nc.tensor.matmul(
                    psk[:], wkt[:, k, 0:P], xt[:, 0, k, :],
                    start=(k == 0), stop=(k == NKC - 1),
                    skip_group_check=True,
                )
            psq = ypp.tile([P, 512], F32, tag="yp")
            for k in range(NKC):
                nc.tensor.matmul(
                    psq[:], wqt[:, k, 0:P], xt[:, 0, k, :],
                    start=(k == 0), stop=(k == NKC - 1),
                    skip_group_check=True,
                )
            nc.vector.tensor_scalar_add(kt[:, 0, 0:512], psk[:], bk[:, 0:1])
            nc.vector.tensor_scalar_add(qt[:, 0, 0:512], psq[:], bq[:, 0:1])

            T = lambda f, *a: (lambda: f(*a))
            # pair (0,0): V projection rides in every slot; remaining kt/qt
            # blocks arrive as half-chains spread over adjacent slots so no
            # single slot overloads the in-order PE and starves the exp
            # stream
            s00 = {ks: [T(proj_v, ks)] for ks in range(NST)}
            s00[1].append(T(proj_qk_h, kt, wkt, bk, 0, 1, 0))
            s00[2].append(T(proj_qk_h, kt, wkt, bk, 0, 1, 1))
            s00[5].append(T(proj_qk_h, kt, wkt, bk, 0, 2, 0))
            s00[6].append(T(proj_qk_h, kt, wkt, bk, 0, 2, 1))
            s00[8].append(T(proj_qk_h, kt, wkt, bk, 0, 3, 0))
            s00[9].append(T(proj_qk_h, kt, wkt, bk, 0, 3, 1))
            s00[10].append(T(proj_qk_h, kt, wkt, bk, 1, 0, 0))
            s00[11].append(T(proj_qk_h, kt, wkt, bk, 1, 0, 1))
            s00[13].append(T(proj_qk_h, qt, wqt, bq, 1, 0, 0))
            s00[14].append(T(proj_qk_h, qt, wqt, bq, 1, 0, 1))
            strips(0, 0, s00)

            # pair (0,1): normalize (0,0) once its last attnV has been
            # emitted (lagged to slots 0-3), finish kt e1, start qt blocks
            s01 = {
                0: [T(proj_qk_h, kt, wkt, bk, 1, 1, 0)],
                1: [T(proj_qk_h, kt, wkt, bk, 1, 1, 1)],
                2: [T(proj_qk_h, kt, wkt, bk, 1, 2, 0)],
                3: [T(proj_qk_h, kt, wkt, bk, 1, 2, 1)],
                4: [T(trans_dn, 0, 0, 0)],
                5: [T(trans_dn, 0, 0, 1)],
                6: [T(trans_mul, 0, 0)],
                7: [T(proj_qk_h, kt, wkt, bk, 1, 3, 0)],
                8: [T(proj_qk_h, kt, wkt, bk, 1, 3, 1)],
                9: [T(proj_qk_h, qt, wqt, bq, 0, 1, 0)],
                10: [T(proj_qk_h, qt, wqt, bq, 0, 1, 1)],
                11: [T(proj_qk_h, qt, wqt, bq, 1, 1, 0)],
                12: [T(proj_qk_h, qt, wqt, bq, 1, 1, 1)],
            }
            strips(0, 1, s01)

            # steady state: each pair normalizes the previous pair at slots
            # 2-3; the y tiles of q-block qb-1 are spread over BOTH pairs of
            # qb so heavy and light slots average out
            for qb in range(1, NQB):
                st0 = (qb - 1) * 4
                s0 = {
                    4: [T(trans_dn, qb - 1, 1, 0)],
                    5: [T(trans_dn, qb - 1, 1, 1)],
                    6: [T(trans_mul, qb - 1, 1)],
                    8: [T(ytile, st0)],
                    12: [T(ytile, st0 + 1)],
                }
                if qb == 1:
                    s0[9] = [T(proj_qk_h, qt, wqt, bq, 1, 2, 0)]
                    s0[10] = [T(proj_qk_h, qt, wqt, bq, 1, 2, 1)]
                    s0[13] = [T(proj_qk_h, qt, wqt, bq, 0, 2, 0)]
                    s0[14] = [T(proj_qk_h, qt, wqt, bq, 0, 2, 1)]
                strips(qb, 0, s0)
                s1 = {
                    4: [T(trans_dn, qb, 0, 0)],
                    5: [T(trans_dn, qb, 0, 1)],
                    6: [T(trans_mul, qb, 0)],
                    8: [T(ytile, st0 + 2)],
                    12: [T(ytile, st0 + 3)],
                }
                if qb == 1:
                    s1[9] = [T(proj_qk_h, qt, wqt, bq, 1, 3, 0)]
                    s1[10] = [T(proj_qk_h, qt, wqt, bq, 1, 3, 1)]
                    s1[13] = [T(proj_qk_h, qt, wqt, bq, 0, 3, 0)]
                    s1[14] = [T(proj_qk_h, qt, wqt, bq, 0, 3, 1)]
                strips(qb, 1, s1, lag=LAG_LAST if qb == NQB - 1 else LAG)

            # tail: drain the last two lagged strips so head A's normalize
            # starts as soon as ITS last attnV lands (head B's final attnV
            # overlaps the head-A DVE chain); denominator copies go via the
            # now-idle scalar engine so the vector engine only runs
            # recip+mul; final y tiles run out of the freed attnV psum banks
            # tail: per-head normalize so head A's recip/mul overlap head
            # B's final attnV; denominator copies via the now-idle scalar
            # engine
            def tail_trans(hp):
                qb, p = NQB - 1, 1
                par = (2 * qb + p) % 2
                q0 = qb * 512
                av = av_store[(qb, p)][hp]
                r = hp * (DH // 2)
                rb_ps = ypp.tile([DH, 512], F32, tag="yp")
                nc.tensor.matmul(
                    rb_ps[:],
                    ones2[r : r + 1, hp * DH : (hp + 1) * DH],
                    dnp[r : r + 1, par],
                )
                rb = rbp.tile([DH, 512], F32, tag="rb")
                nc.vector.reciprocal_approx_fast(rb[:], rb_ps[:])
                nc.vector.tensor_mul(
                    at[hp * DH : (hp + 1) * DH, p, q0 : q0 + 512],
                    av[0:DH, :],
                    rb[:],
                )

            e15 = pend.pop(0)
            assert not pend
            emit_head(*e15, 0)
            trans_dn(NQB - 1, 1, 0, dn_scalar=True)
            emit_head(*e15, 1)
            tail_trans(0)
            trans_dn(NQB - 1, 1, 1, dn_scalar=True)
            tail_trans(1)
            for st in range((NQB - 1) * 4, NQB * 4):
                ytile(st, pool=avp, tag="av", scalar_copy=True)

    nc.finalize()
    return nc


def _get_nc():
    if "nc" not in _CACHE:
        _CACHE["nc"] = _build_nc()
    return _CACHE["nc"]


def _prep_inputs(X, Wq, bq, Wk, bk, Wv, bv, Wo, bo):
    F16 = np.float16
    f = lambda a: np.asarray(a, dtype=np.float32)
    X, Wq, bq, Wk, bk, Wv, bv, Wo, bo = map(f, (X, Wq, bq, Wk, bk, Wv, bv, Wo, bo))
    B = X.shape[0]
    scale = np.float32(1.0 / np.sqrt(DH))
    def arr_x(a):  # [D,S] -> [sb, p, o, s]
        return np.ascontiguousarray(
            a.reshape(NKC, P, 4, 512).transpose(2, 1, 0, 3).astype(F16))

    def arr_w(a, o):  # [D_in, E] -> [p, o, e]
        return np.ascontiguousarray(
            a.reshape(o, P, -1).transpose(1, 0, 2).astype(F16))

    XT = [arr_x(X[b].T) for b in range(B)]
    in_maps = []
    for c in range(2 * B):
        b, hg = divmod(c, 2)
        sl = slice(hg * EL, (hg + 1) * EL)
        in_maps.append(
            {
                "XT": XT[b],
                "WQT": arr_w((Wq[sl] * scale).T, NKC),
                "WKT": arr_w(Wk[sl].T, NKC),
                "WVT": arr_w(Wv[sl].T, NKC),
                "WOT": arr_w(Wo[:, sl].T, 2),
                "BQ": np.ascontiguousarray(bq[sl] * scale),
                "BK": np.ascontiguousarray(bk[sl]),
                "BVB": np.ascontiguousarray(np.tile(bv[sl], (P, 1))),
            }
        )
    return in_maps, bo, B


def run(inputs, trace=False, trace_cores=None):
    """Run the kernel; returns (Y_full, exec_time_ns or None)."""
    from concourse.bass_utils import run_bass_kernel_spmd

    in_maps, bo, B = _prep_inputs(**inputs)
    nc = _get_nc()
    kw = {}
    if trace:
        kw = dict(trace=True, trace_cores=trace_cores or list(range(2 * B)))
    res = run_bass_kernel_spmd(nc, in_maps, list(range(2 * B)), **kw)
    Y = np.stack(
        [
            res.results[2 * b]["Y"].astype(np.float32)
            + res.results[2 * b + 1]["Y"].astype(np.float32)
            + bo
            for b in range(B)
        ]
    )
    return Y, getattr(res, "exec_time_ns", None)


def kernel(X, Wq, bq, Wk, bk, Wv, bv, Wo, bo):
    Y, _ = run(
        dict(X=X, Wq=Wq, bq=bq, Wk=Wk, bk=bk, Wv=Wv, bv=bv, Wo=Wo, bo=bo)
    )
    return Y

